# revision 12
# baseline (speedup 1.0000x reference)
"""Trainium2 Bass kernel for GaussianRenderer (BEV gaussian splatting).

Strategy (8 cores): data-parallel over batch (B=2) x 4 row-bands of the
200x200 image (50 rows each). Within a core, the band is tiled into 20
pixel tiles of 25x20 = 500 pixels. Gaussians are culled per tile on the
host (conservative per-gaussian radius, order preserved, padded to chunks
of 128). Per chunk the device computes:
  q' = quadratic form - ln(opacity)      (PE matmul vs basis [1,x,y,x2,xy,y2])
  e = exp(-q')                           (ACT, from PSUM)
  alpha = min(e, 0.99) * (e >= 1/255)    (DVE ts + scalar_tensor_tensor)
  logom = ln(1 - alpha)                  (ACT)
  s = excl-cumsum(logom) + carry         (PE: strict-triangular + all-ones matmuls)
  w = exp(s)                             (ACT)
  wgt = alpha * w                        (DVE)
  acc += feats^T @ wgt                   (PE, PSUM accumulate across chunks)
Front-to-back compositing order is preserved because per-tile culled lists
keep the original gaussian order and excluded gaussians have alpha < 1/255
(exactly zeroed by the reference's mask too).
"""

import numpy as np

H = W = 200
SH = SW = 2.0
THRESH = 0.05
LOWPASS = 0.3
ALPHA_MIN = 1.0 / 255.0
ALPHA_MAX = 0.99

TILE_H, TILE_W = 25, 20
N_PIX = TILE_H * TILE_W          # 500
BAND_H = 50                      # rows per core
N_CORES = 8
TILES_Y = BAND_H // TILE_H       # 2
TILES_X = W // TILE_W            # 10
N_TILES = TILES_Y * TILES_X      # 20

_cache = {}


def _prep_gaussians(means, cov6, opac):
    """Screen-space params + conservative cull radius (f64)."""
    means = means.astype(np.float64)
    cov6 = cov6.astype(np.float64)
    opac = opac.astype(np.float64)
    px = -SH * means[:, 1] + W / 2.0
    py = -SW * means[:, 0] + H / 2.0
    a = SH * SH * cov6[:, 3] + LOWPASS
    b = SH * SW * cov6[:, 1]
    c = SW * SW * cov6[:, 0] + LOWPASS
    det = a * c - b * b
    valid = (det > 0) & (opac > THRESH)
    inv_det = np.where(det > 0, 1.0 / np.maximum(det, 1e-300), 0.0)
    cA = c * inv_det
    cB = -b * inv_det
    cC = a * inv_det
    lam_max = 0.5 * (a + c) + np.sqrt(0.25 * (a - c) ** 2 + b * b)
    ln_term = np.log(np.maximum(opac, 1e-30) / ALPHA_MIN)
    radius = np.sqrt(2.0 * lam_max * np.maximum(ln_term, 0.0)) + 0.75
    radius = np.where(valid, radius, -1.0)
    return px, py, cA, cB, cC, radius


def _tile_coef(px, py, cA, cB, cC, opac, idx, x_c, y_c):
    """Coefficients of q(x,y) - ln(o) in basis [1,x,y,x2,xy,y2], tile-centered."""
    dpx = px[idx] - x_c
    dpy = py[idx] - y_c
    A, B, C = cA[idx], cB[idx], cC[idx]
    lno = np.log(opac[idx].astype(np.float64))
    c0 = 0.5 * A * dpx ** 2 + B * dpx * dpy + 0.5 * C * dpy ** 2 - lno
    c1 = -(A * dpx + B * dpy)
    c2 = -(B * dpx + C * dpy)
    return np.stack([c0, c1, c2, 0.5 * A, B, 0.5 * C], axis=0)


def _host_shard(features, means3D, cov3D, opacities):
    """Build per-core device inputs. Returns (coef, feat) per core and C."""
    B, h, w, d = features.shape
    P = h * w
    per_sample = []
    for s in range(B):
        feats = features[s].reshape(P, d).astype(np.float32)
        means = means3D[s].reshape(P, 3)
        cov6 = cov3D[s].reshape(P, 6)
        opac = opacities[s].reshape(P)
        px, py, cA, cB, cC, radius = _prep_gaussians(means, cov6, opac)
        per_sample.append((feats, px, py, cA, cB, cC, radius, opac))

    # cull per (core, tile)
    idx_lists = [[] for _ in range(N_CORES)]
    max_g = 1
    for core in range(N_CORES):
        s, band = core // 4, core % 4
        _, px, py, cA, cB, cC, radius, _ = per_sample[s]
        row0 = band * BAND_H
        for ty in range(TILES_Y):
            for tx in range(TILES_X):
                y0, x0 = row0 + ty * TILE_H, tx * TILE_W
                cx = np.clip(px, x0, x0 + TILE_W - 1)
                cy = np.clip(py, y0, y0 + TILE_H - 1)
                dist2 = (px - cx) ** 2 + (py - cy) ** 2
                idx = np.nonzero(dist2 <= radius * radius)[0]
                idx_lists[core].append(idx)
                max_g = max(max_g, len(idx))
    C = (max_g + 127) // 128
    G = C * 128

    d_pad = d  # 32
    coef_all = np.zeros((N_CORES, N_TILES * C, 6, 128), np.float32)
    coef_all[:, :, 0, :] = 100.0  # padding: q'=100 -> e ~ 0
    feat_all = np.zeros((N_CORES, N_TILES * C, 128, d_pad), np.float32)
    for core in range(N_CORES):
        s, band = core // 4, core % 4
        feats, px, py, cA, cB, cC, radius, opac = per_sample[s]
        row0 = band * BAND_H
        for t in range(N_TILES):
            ty, tx = divmod(t, TILES_X)
            y0, x0 = row0 + ty * TILE_H, tx * TILE_W
            x_c = x0 + (TILE_W - 1) / 2.0
            y_c = y0 + (TILE_H - 1) / 2.0
            idx = idx_lists[core][t]
            g = len(idx)
            if g == 0:
                continue
            cf = _tile_coef(px, py, cA, cB, cC, opac, idx, x_c, y_c)
            cf = cf.astype(np.float32)  # (6, g)
            ft = feats[idx]             # (g, d)
            for c in range(C):
                lo, hi = c * 128, min((c + 1) * 128, g)
                if lo >= g:
                    break
                k = t * C + c
                coef_all[core, k, :, : hi - lo] = cf[:, lo:hi]
                feat_all[core, k, : hi - lo, :] = ft[lo:hi]
    return coef_all, feat_all, C, d_pad


def _basis_np():
    yy, xx = np.meshgrid(np.arange(TILE_H), np.arange(TILE_W), indexing="ij")
    xs = (xx.ravel() - (TILE_W - 1) / 2.0).astype(np.float32)
    ys = (yy.ravel() - (TILE_H - 1) / 2.0).astype(np.float32)
    return np.stack([np.ones_like(xs), xs, ys, xs * xs, xs * ys, ys * ys], 0)


def _build_program(C, d):
    import concourse.bass as bass
    from concourse import mybir
    import concourse.tile as tile

    fp32 = mybir.dt.float32
    AF = mybir.ActivationFunctionType
    ALU = mybir.AluOpType

    nc = bass.Bass("TRN2", target_bir_lowering=False, debug=False,
                   num_devices=N_CORES)
    NK = N_TILES * C
    coef_d = nc.dram_tensor("coef", [NK, 6, 256], fp32, kind="ExternalInput").ap()
    feat_d = nc.dram_tensor("feat", [NK, 128, d], fp32, kind="ExternalInput").ap()
    basis_d = nc.dram_tensor("basis", [2, 6, N_PIX], fp32, kind="ExternalInput").ap()
    lext_d = nc.dram_tensor("lext", [128, 128], fp32, kind="ExternalInput").ap()
    ones_d = nc.dram_tensor("ones", [128, 128], fp32, kind="ExternalInput").ap()
    out_d = nc.dram_tensor("out", [d, BAND_H * W], fp32, kind="ExternalOutput").ap()

    with tile.TileContext(nc) as tc:
        with (
            tc.tile_pool(name="consts", bufs=1) as consts,
            tc.tile_pool(name="params", bufs=4) as params,
            tc.tile_pool(name="work", bufs=3) as work,
            tc.tile_pool(name="logoms", bufs=2 * C + 2) as logoms,
            tc.tile_pool(name="outp", bufs=1) as outp,
            tc.tile_pool(name="qp", bufs=2, space="PSUM") as qp,
            tc.tile_pool(name="sp", bufs=2, space="PSUM") as sp,
            tc.tile_pool(name="accp", bufs=2, space="PSUM") as accp,
        ):
            basis_hi = consts.tile([6, N_PIX], fp32, tag="basis_hi")
            nc.sync.dma_start(out=basis_hi, in_=basis_d[0])
            basis_lo = consts.tile([6, N_PIX], fp32, tag="basis_lo")
            nc.sync.dma_start(out=basis_lo, in_=basis_d[1])
            lext_s = consts.tile([128, 128], fp32, tag="lext")
            nc.sync.dma_start(out=lext_s, in_=lext_d)
            ones_s = consts.tile([128, 128], fp32, tag="ones")
            nc.sync.dma_start(out=ones_s, in_=ones_d)

            # Absorb the const-DMA queue waits onto PE via tiny dummy
            # matmuls, so no later matmul needs >1 DMA-queue sem wait
            # (walrus limit on LDWEIGHTS sync commands).
            scratch = qp.tile([2, 4], fp32, tag="scratch")
            for cst in (basis_hi, basis_lo, lext_s, ones_s):
                nc.tensor.matmul(scratch, lhsT=cst[0:1, 0:2],
                                 rhs=cst[0:1, 0:4], start=True, stop=True)

            out_s = outp.tile([d, BAND_H, W], fp32, tag="out")

            for t in range(N_TILES):
                ty, tx = divmod(t, TILES_X)
                acc = accp.tile([d, N_PIX], fp32, tag="acc")
                lo_list = []
                for c in range(C):
                    k = t * C + c
                    coef_s = params.tile([6, 256], fp32, tag="coef")
                    nc.sync.dma_start(out=coef_s, in_=coef_d[k])
                    feat_s = params.tile([128, d], fp32, tag="feat")
                    nc.sync.dma_start(out=feat_s, in_=feat_d[k])

                    q = qp.tile([128, N_PIX], fp32, tag="q")
                    nc.tensor.matmul(q, lhsT=coef_s[:, 0:128], rhs=basis_hi,
                                     start=True, stop=False,
                                     skip_group_check=True)
                    nc.tensor.matmul(q, lhsT=coef_s[:, 0:128], rhs=basis_lo,
                                     start=False, stop=False,
                                     skip_group_check=True)
                    nc.tensor.matmul(q, lhsT=coef_s[:, 128:256], rhs=basis_hi,
                                     start=False, stop=True,
                                     skip_group_check=True)
                    e = work.tile([128, N_PIX], fp32, tag="e")
                    nc.scalar.activation(e, q, AF.Exp, scale=-1.0)
                    am = work.tile([128, N_PIX], fp32, tag="am")
                    nc.vector.tensor_scalar(am, e, ALPHA_MAX, None, ALU.min)
                    alpha = work.tile([128, N_PIX], fp32, tag="alpha")
                    nc.vector.scalar_tensor_tensor(
                        alpha, e, ALPHA_MIN, am, ALU.is_ge, ALU.mult)
                    logom = logoms.tile([128, N_PIX], fp32, tag="logom")
                    nc.scalar.activation(logom, alpha, AF.Ln,
                                         bias=1.0, scale=-1.0)

                    s_ps = sp.tile([128, N_PIX], fp32, tag="s")
                    nc.tensor.matmul(s_ps, lhsT=lext_s, rhs=logom,
                                     start=True, stop=(c == 0),
                                     skip_group_check=True)
                    for lo_prev in lo_list:
                        nc.tensor.matmul(s_ps, lhsT=ones_s, rhs=lo_prev,
                                         start=False,
                                         stop=(lo_prev is lo_list[-1]),
                                         skip_group_check=True)
                    lo_list.append(logom)

                    wt = work.tile([128, N_PIX], fp32, tag="wt")
                    nc.scalar.activation(wt, s_ps, AF.Exp)
                    wgt = work.tile([128, N_PIX], fp32, tag="wgt")
                    nc.vector.tensor_tensor(wgt, alpha, wt, ALU.mult)

                    nc.tensor.matmul(acc, lhsT=feat_s, rhs=wgt,
                                     start=(c == 0), stop=(c == C - 1),
                                     skip_group_check=True)

                dst = out_s[:, ty * TILE_H:(ty + 1) * TILE_H,
                            tx * TILE_W:(tx + 1) * TILE_W]
                acc3 = acc.rearrange("p (a b) -> p a b", a=TILE_H)
                nc.vector.tensor_copy(dst, acc3)

            nc.sync.dma_start(
                out=out_d, in_=out_s.rearrange("p a b -> p (a b)"))
    _split_waits(nc)
    return nc


def _split_waits(nc):
    """walrus (bass2jax path) allows only ONE sync wait per instruction.
    Move extra waits onto same-engine NoOps inserted just before."""
    from concourse import mybir
    n = 0
    for f in nc.m.functions:
        for bb in f.blocks:
            new_insts = []
            for inst in bb.instructions:
                si = inst.sync_info
                if si is not None and si.on_wait and len(si.on_wait) > 1:
                    for w in si.on_wait[:-1]:
                        n += 1
                        new_insts.append(mybir.InstNoOp(
                            name=f"I-sw{n}", ins=[], outs=[],
                            engine=inst.engine,
                            sync_info=mybir.SyncInfo(
                                on_wait=[w], on_update=[]),
                        ))
                    inst.sync_info = mybir.SyncInfo(
                        on_wait=list(si.on_wait[-1:]),
                        on_update=list(si.on_update))
                new_insts.append(inst)
            bb.instructions = new_insts


def _split_hi_lo(x):
    """x (f32) -> (hi, lo) f32 with hi exactly bf16-representable."""
    import ml_dtypes
    hi = x.astype(ml_dtypes.bfloat16).astype(np.float32)
    return hi, (x - hi).astype(np.float32)


def build_in_maps(coef_all, feat_all):
    basis = _basis_np()
    b_hi, b_lo = _split_hi_lo(basis)
    basis2 = np.stack([b_hi, b_lo], 0)                    # (2, 6, 500)
    c_hi, c_lo = _split_hi_lo(coef_all)                   # (NC, NK, 6, 128)
    coef2 = np.concatenate([c_hi, c_lo], axis=-1)         # (NC, NK, 6, 256)
    lext = np.triu(np.ones((128, 128), np.float32), 1)
    ones = np.ones((128, 128), np.float32)
    return [{
        "coef": coef2[core],
        "feat": feat_all[core],
        "basis": basis2,
        "lext": lext,
        "ones": ones,
    } for core in range(N_CORES)]


def kernel(features, means3D, cov3D, opacities):
    features = np.asarray(features, np.float32)
    means3D = np.asarray(means3D, np.float32)
    cov3D = np.asarray(cov3D, np.float32)
    opacities = np.asarray(opacities, np.float32)
    B, h, w, d = features.shape

    coef_all, feat_all, C, d_pad = _host_shard(
        features, means3D, cov3D, opacities)

    key = (C, d_pad)
    if key not in _cache:
        _cache[key] = _build_program(C, d_pad)
    nc = _cache[key]

    in_maps = build_in_maps(coef_all, feat_all)

    from concourse.bass_utils import run_bass_kernel_spmd
    res = run_bass_kernel_spmd(nc, in_maps, list(range(N_CORES)))

    bev = np.zeros((B, d, H, W), np.float32)
    for core in range(N_CORES):
        s, band = core // 4, core % 4
        o = res.results[core]["out"].reshape(d, BAND_H, W)
        bev[s, :, band * BAND_H:(band + 1) * BAND_H, :] = o

    mask = (opacities[..., 0] > THRESH).astype(np.float32)
    count = np.float32(np.mean(np.sum(mask, axis=1)))
    return bev, count
